# revision 1
# baseline (speedup 1.0000x reference)
"""Bass/Tile kernel for nn_Attention_49959059587521 on 8 TRN2 NeuronCores.

Math per (batch b, head h), with Q,K,V,Q2,K2 = [2048, 64] slices:
    S    = (Q @ K^T) * 0.125                    # [2048, 2048]
    P    = softmax(S, axis=-1)
    gate = sigmoid((Q2 @ sum_n(K2)) * 0.125)    # [2048]
    out  = (P * gate[:, None]) @ V              # [2048, 64]

Sharding: 32 (b, h) pairs over 8 cores -> core i handles b = i//2 and the 4
heads h in [4*(i%2), 4*(i%2)+4), i.e. the channel slice [256*(i%2), +256).
No cross-core communication.

Per-core algorithm (fully on device):
  - S^T[k, q] = K^T(stationary) x Q^T(moving) via bf16 matmuls. Heads are
    processed in stacked pairs so every matmul contracts over K=128
    partitions (K=64 streams at half rate and K-switches stall the PE);
    each head's K^T weights are zero-padded on the other head's 64
    partitions.
  - exp fused on ScalarE reading PSUM directly (scale=0.125 via free affine);
    no max-subtraction needed: logits are ~N(0,1), |S| < ~7, exp is safe in f32.
  - O^T = V'^T @ E accumulated in PSUM over the 16 k-tiles, where V' = [V; ones]
    so row 64 of O^T is the softmax denominator (free rowsum).
  - V' is filled directly by casting DMAs (gpsimd SW-DGE casts f32->bf16 in
    flight), one 64-column slice per head: no staging tile, no DVE copies,
    and only head 0's slice is needed in the startup window.
  - gate computed as 1/(1+exp(-z*scale)) (exp table only; inf-safe); its
    k2 column-sum runs as a DVE halving-add tree + one fp32 ones-matmul on
    the PE for the partition-allreduce (result replicated over partitions).
  - O^T 128-column blocks PE-transposed (bf16) back to [q, d]; the output
    scale fuses (gate * 1/rowsum) in one two-scalar VectorE instruction.

Scheduling notes (the ScalarE exp stream, 128 x ~1.07us, is the hard floor;
the startup is HBM-arrival-bound across all 8 cores sharing the device).
The Tile scheduler fixes each engine's instruction order at compile time
from its own latency model, so emission order/priority AND queue assignment
must match real data arrival, or queues head-of-line block:
  - identity built on GpSimd BEFORE the DMA triggers; all four K^T
    zero-half memsets lead the DVE queue.
  - Pool SW-DGE trigger order is strictly by first-need: q0 k0 v0 k1 k2 v1
    k3 v2 v3 (k front-loaded: k group g gates st(4g); v slice j=0 gates the
    first accumulates). q1 rides the parallel SP HW-DGE channel. q2/k2 are
    gated behind tile_wait_until so they neither steal HBM bandwidth from
    the hot loads nor mislead the scheduler about the gate chain's timing
    (a 2MB k2 on a HW-DGE queue once serialized the whole DVE order).
  - both head-pairs' transposes are emitted per cast-group so all of
    stage B completes in the DMA-bound startup window.
  - phase2 (O^T transpose + rowsum/gate scale + store) is emitted one unit
    per t-slot INTO the next half's t-loop: emitted-earlier work outranks
    the live stream the moment it becomes ready, so a block-emitted phase2
    burst used to stall the exp stream ~1.5us at every half boundary.
  - the last half drains at per-tile grain with progressively finer output
    DMAs to shrink the tail.
Note: the axon-shared TRN2 shows a bimodal device clock state (~19%:
exp 1060ns vs 1293ns per tile, uniform across every engine, minutes-long
windows) — cross-run comparisons are only valid within one mode.
"""

import functools
from contextlib import ExitStack

import numpy as np

import concourse.mybir as mybir
import concourse.tile as tile
from concourse import bacc, bass_utils
from concourse.masks import make_identity

F32 = mybir.dt.float32

B, NT, C, H = 4, 2048, 512, 8
HD = 64
SCALE = HD ** -0.5  # 0.125
P = 128
NO = NT // P            # 16 n-tiles
NH4 = 4                 # heads per core
CW = NH4 * HD           # 256 channels per core
NHALF = 2               # q processed in two halves of 1024
QH = NT // NHALF        # 1024
BF16 = mybir.dt.bfloat16
MM_DT = BF16            # dtype of matmul operands (qT/kT/V'/E)
U32 = mybir.dt.uint32


def _build(use_sigmoid: bool):
    nc = bacc.Bacc("TRN2", target_bir_lowering=False)
    q_d = nc.dram_tensor("q", [NT, CW], F32, kind="ExternalInput")
    k_d = nc.dram_tensor("k", [NT, CW], F32, kind="ExternalInput")
    v_d = nc.dram_tensor("v", [NT, CW], F32, kind="ExternalInput")
    if use_sigmoid:
        q2_d = nc.dram_tensor("q2", [NT, CW], F32, kind="ExternalInput")
        k2_d = nc.dram_tensor("k2", [NT, CW], F32, kind="ExternalInput")
    out_d = nc.dram_tensor("out", [NT, CW], F32, kind="ExternalOutput")

    with tile.TileContext(nc) as tc, ExitStack() as ctx:
        singles = ctx.enter_context(tc.tile_pool(name="singles", bufs=1))
        tpool = ctx.enter_context(tc.tile_pool(name="tp", bufs=2))
        epool = ctx.enter_context(tc.tile_pool(name="ep", bufs=4))
        opool = ctx.enter_context(tc.tile_pool(name="op", bufs=2))
        # PSUM: st 2x[128,1024] = 4 banks, acc 1x[65,1024] = 2 banks,
        # tr 2x[<=128,<=512] = 2 banks. Total 8 banks. (A shared 3-slot
        # st/tr rotation measured worse: each st allocation then waits on a
        # preceding transpose tile's phase2 consumers.)
        ps_st = ctx.enter_context(tc.tile_pool(name="ps_st", bufs=2, space="PSUM"))
        ps_ac = ctx.enter_context(tc.tile_pool(name="ps_ac", bufs=1, space="PSUM"))
        ps_tr = ctx.enter_context(tc.tile_pool(name="ps_tr", bufs=2, space="PSUM"))

        def tr_tile(shape, dtype=F32):
            return ps_tr.tile(shape, dtype, tag="ptr", name="ptr")

        # ---- identity (bf16, used by stage B and phase2 transposes)
        # FIRST on GpSimd: PE transposes need it ~13us in.
        ident_b = singles.tile([P, P], BF16)
        make_identity(nc, ident_b)

        # ---- input tiles ([n, c] -> [p, o, c] tiling) ----
        q_sb = singles.tile([P, NO, CW], F32, name="q_sb", tag="q_sb")
        k_sb = singles.tile([P, NO, CW], F32, name="k_sb", tag="k_sb")
        # V' = [V | ones] per head, bf16, filled directly by casting DMAs
        v1r = singles.tile([P, NO, NH4, HD + 1], MM_DT)
        q_src = q_d.ap().rearrange("(o p) c -> p o c", p=P)
        k_src = k_d.ap().rearrange("(o p) c -> p o c", p=P)
        v_src = v_d.ap().rearrange("(o p) c -> p o c", p=P)

        def g_sl(g):
            return slice(4 * g, 4 * (g + 1))

        # Pool SW-DGE triggers, strictly first-needed first: the startup is
        # HBM-arrival-bound, so the first half's needs (q01, k*, v slice 0)
        # must not share bandwidth with anything needed later. v slices for
        # heads 1-3 are needed only at ~57/92/127us.
        nc.gpsimd.dma_start(q_sb[:, g_sl(0), :], q_src[:, g_sl(0), :])
        nc.gpsimd.dma_start(k_sb[:, g_sl(0), :], k_src[:, g_sl(0), :])
        nc.gpsimd.dma_start(
            v1r[:, :, 0, 0:HD], v_src[:, :, 0:HD]
        )
        nc.gpsimd.dma_start(k_sb[:, g_sl(1), :], k_src[:, g_sl(1), :])
        nc.gpsimd.dma_start(k_sb[:, g_sl(2), :], k_src[:, g_sl(2), :])
        nc.gpsimd.dma_start(k_sb[:, g_sl(3), :], k_src[:, g_sl(3), :])
        nc.gpsimd.dma_start(q_sb[:, g_sl(2), :], q_src[:, g_sl(2), :])
        nc.gpsimd.dma_start(q_sb[:, g_sl(3), :], q_src[:, g_sl(3), :])
        nc.gpsimd.dma_start(
            v1r[:, :, 1, 0:HD], v_src[:, :, HD : 2 * HD]
        )
        nc.gpsimd.dma_start(
            v1r[:, :, 2, 0:HD], v_src[:, :, 2 * HD : 3 * HD]
        )
        nc.gpsimd.dma_start(
            v1r[:, :, 3, 0:HD], v_src[:, :, 3 * HD : 4 * HD]
        )
        # gate inputs pinned in sim time, behind all hot loads
        if use_sigmoid:
            q2_sb = singles.tile([P, NO, CW], F32, name="q2_sb", tag="q2_sb")
            k2_sb = singles.tile([P, NO, CW], F32, name="k2_sb", tag="k2_sb")
            q2_src = q2_d.ap().rearrange("(o p) c -> p o c", p=P)
            k2_src = k2_d.ap().rearrange("(o p) c -> p o c", p=P)
            # single whole-tensor triggers: the scheduler's serial
            # per-queue transfer model then predicts k2 at ~41us (close to
            # the ~37us reality) instead of ~60us, so the gate chain lands
            # at the right place in the fixed DVE order
            with tc.tile_wait_until(0.026):
                nc.gpsimd.dma_start(q2_sb, q2_src)
                nc.gpsimd.dma_start(k2_sb, k2_src)

        # SP HW-DGE channel in parallel: q group 1 (st(0) streams q rows
        # 0:1024).
        nc.sync.dma_start(q_sb[:, g_sl(1), :], q_src[:, g_sl(1), :])

        kTz_all = []
        for jp in range(NH4 // 2):
            kTza = tpool.tile([P, NT], MM_DT, tag="kTza", name="kTza")
            kTzb = tpool.tile([P, NT], MM_DT, tag="kTzb", name="kTzb")
            kTz_all.extend([kTza, kTzb])
        # V' ones column for all heads in one memset.
        nc.gpsimd.memset(v1r[:, :, :, HD : HD + 1], 1.0)
        if use_sigmoid:
            # all-ones stationary for the k2 partition-allreduce matmul
            ones_sb = singles.tile([P, P], F32)
            nc.gpsimd.memset(ones_sb, 1.0)

        # DVE queue head: all four K^T zero-half memsets (DVE is idle until
        # the first q tiles land; the first memset gates st(0)).
        nc.vector.memset(kTz_all[0][HD:P, :].bitcast(U32), 0)
        nc.vector.memset(kTz_all[1][0:HD, :].bitcast(U32), 0)
        nc.vector.memset(kTz_all[2][HD:P, :].bitcast(U32), 0)
        nc.vector.memset(kTz_all[3][0:HD, :].bitcast(U32), 0)

        qbf = singles.tile([P, NO, CW], BF16)
        kbf = singles.tile([P, NO, CW], BF16)
        qT2s = [
            tpool.tile([P, NT], MM_DT, tag="qT2", name="qT2") for _ in range(2)
        ]

        # ---- stage B: stacked transposes for BOTH head pairs per cast
        # group: qT2[jp] [128, 2048] holds heads 2jp (partitions 0:64) and
        # 2jp+1 (64:128); kT is split into two zero-padded weight tensors so
        # the logit matmuls contract over the full 128 partitions. All of it
        # runs in the DMA-bound startup window.
        for g in range(NO // 4):
            gsl = g_sl(g)
            nc.vector.tensor_copy(qbf[:, gsl, :], q_sb[:, gsl, :])
            nc.vector.tensor_copy(kbf[:, gsl, :], k_sb[:, gsl, :])
            for jp in range(2):
                cp = 2 * HD * jp
                tp = tr_tile([P, 4 * P], BF16)
                for u in range(4):
                    o = 4 * g + u
                    nc.tensor.transpose(
                        tp[:, P * u : P * (u + 1)], qbf[:, o, cp : cp + P], ident_b
                    )
                nc.vector.tensor_copy(qT2s[jp][:, 4 * P * g : 4 * P * (g + 1)], tp)
                tp2 = tr_tile([P, 4 * P], BF16)
                for u in range(4):
                    o = 4 * g + u
                    nc.tensor.transpose(
                        tp2[:, P * u : P * (u + 1)], kbf[:, o, cp : cp + P], ident_b
                    )
                nc.vector.tensor_copy(
                    kTz_all[2 * jp][0:HD, 4 * P * g : 4 * P * (g + 1)], tp2[0:HD]
                )
                nc.vector.tensor_copy(
                    kTz_all[2 * jp + 1][HD:P, 4 * P * g : 4 * P * (g + 1)], tp2[HD:P]
                )

        # ---- gate factors for all heads (emitted at head-1 priority) ----
        def emit_gate_all():
            # o-sum as a contiguous binary-halving add tree
            t8 = singles.tile([P, 8, CW], F32)
            nc.vector.tensor_add(t8, k2_sb[:, 0:8, :], k2_sb[:, 8:16, :])
            t4 = singles.tile([P, 4, CW], F32)
            nc.vector.tensor_add(t4, t8[:, 0:4, :], t8[:, 4:8, :])
            t2 = singles.tile([P, 2, CW], F32)
            nc.vector.tensor_add(t2, t4[:, 0:2, :], t4[:, 2:4, :])
            k2o = singles.tile([P, CW], F32)
            nc.vector.tensor_add(k2o, t2[:, 0, :], t2[:, 1, :])
            # partition-allreduce via a single ones-matmul: out[m, c] =
            # sum_p k2o[p, c], replicated over all 128 output partitions
            # (fp32, 4 cyc/col -> ~0.4us; exact).
            k2b_ps = tr_tile([P, CW])
            nc.tensor.matmul(k2b_ps, ones_sb, k2o, start=True, stop=True)
            k2b_sb = singles.tile([P, CW], F32)
            nc.vector.tensor_copy(k2b_sb, k2b_ps)
            # all heads in two DVE passes
            zt = opool.tile([P, NO, CW], F32, tag="zt", name="zt")
            nc.vector.tensor_mul(
                zt, q2_sb, k2b_sb[:, None, :].to_broadcast((P, NO, CW))
            )
            z_all = singles.tile([P, NO, NH4], F32)
            nc.vector.reduce_sum(
                out=z_all,
                in_=zt.rearrange("p o (j c) -> p o j c", j=NH4),
                axis=mybir.AxisListType.X,
            )
            eg_all = singles.tile([P, NO, NH4], F32)
            nc.scalar.activation(
                eg_all, z_all, mybir.ActivationFunctionType.Exp, scale=-SCALE
            )
            nc.vector.tensor_scalar_add(eg_all, eg_all, 1.0)
            g_t = singles.tile([P, NO, NH4], F32)
            nc.vector.reciprocal(g_t, eg_all)
            return g_t

        out_ap3 = out_d.ap().rearrange("(o p) c -> p o c", p=P)
        gte_all = None

        def phase2_units(j, h, ot_sb, last=False):
            """One closure per O^T 128-column block: transpose + scale +
            (chunked) store. Emitted one per t-slot inside the NEXT half's
            t-loop so they never outrank the live exp stream."""
            ch = HD * j
            state = {}
            bounds = [4, 6, 7, 8] if last else [4, 8]

            def mk(u):
                def emit():
                    if u == 0:
                        state["obuf"] = opool.tile(
                            [P, QH // P, HD], F32, tag="obuf", bufs=4, name="obuf"
                        )
                    obuf = state["obuf"]
                    i = (QH // P) * h + u
                    tr = tr_tile([P, HD + 1], MM_DT)
                    nc.tensor.transpose(
                        tr, ot_sb[:, P * u : P * (u + 1)], ident_b[: HD + 1, : HD + 1]
                    )
                    rcp = opool.tile([P, 1], F32, tag="rcp", name="rcp")
                    nc.vector.reciprocal(rcp, tr[:, HD : HD + 1])
                    if use_sigmoid:
                        # (O^T/rowsum)*gate in one two-scalar instruction
                        nc.vector.tensor_scalar(
                            obuf[:, u, :],
                            tr[:, 0:HD],
                            rcp,
                            gte_all[:, i, j : j + 1],
                            mybir.AluOpType.mult,
                            mybir.AluOpType.mult,
                        )
                    else:
                        nc.vector.tensor_scalar_mul(obuf[:, u, :], tr[:, 0:HD], rcp)
                    if u + 1 in bounds:
                        c0 = 0 if u + 1 == bounds[0] else bounds[bounds.index(u + 1) - 1]
                        nc.sync.dma_start(
                            out_ap3[:, 8 * h + c0 : 8 * h + u + 1, ch : ch + HD],
                            obuf[:, c0 : u + 1, :],
                        )

                return emit

            return [mk(u) for u in range(QH // P)]

        # ---- main loop: per (head, half): 16x [st matmuls -> exp -> acc
        # matmuls] with deferred phase2 units drip-fed into the t-slots.
        pending = []
        deferred0 = []
        for j in range(NH4):  # local head
            jp, jj = divmod(j, 2)
            qT2 = qT2s[jp]
            kTz = kTz_all[2 * jp + jj]
            if j == 1:
                for dh, dot in deferred0:
                    pending.extend(phase2_units(0, dh, dot))
                deferred0 = []
            for h in range(NHALF):  # q half
                if use_sigmoid and j == 0 and h == 1:
                    # gate chain at (0,h1) priority: its ~13us of DVE work
                    # fills this half's otherwise-empty DVE window (the drip
                    # units only start at head 1) and eg is ready ~48us,
                    # well before the first phase2 fac at ~58us.
                    gte_all = emit_gate_all()
                last = j == NH4 - 1 and h == NHALF - 1
                q0 = QH * h
                acc = ps_ac.tile([HD + 1, QH], F32, tag="pac")
                for t in range(NO):
                    st = ps_st.tile([P, QH], F32, tag="pst")
                    for s2 in range(QH // 512):
                        nc.tensor.matmul(
                            st[:, 512 * s2 : 512 * (s2 + 1)],
                            kTz[:, P * t : P * (t + 1)],
                            qT2[:, q0 + 512 * s2 : q0 + 512 * (s2 + 1)],
                            start=True,
                            stop=True,
                        )
                    et = epool.tile([P, QH], MM_DT, tag="et")
                    nc.scalar.activation(
                        et, st, mybir.ActivationFunctionType.Exp, scale=SCALE
                    )
                    for s2 in range(QH // 512):
                        nc.tensor.matmul(
                            acc[:, 512 * s2 : 512 * (s2 + 1)],
                            v1r[:, t, j, :],
                            et[:, 512 * s2 : 512 * (s2 + 1)],
                            start=(t == 0),
                            stop=(t == NO - 1),
                        )
                    if t >= 2 and pending:
                        pending.pop(0)()

                ot_sb = opool.tile([HD + 1, QH], MM_DT, tag="ot", bufs=4, name="ot_sb")
                if last:
                    # per-u copies so the drain pipeline starts immediately
                    for u in range(QH // P):
                        nc.vector.tensor_copy(
                            ot_sb[:, P * u : P * (u + 1)],
                            acc[:, P * u : P * (u + 1)],
                        )
                else:
                    nc.vector.tensor_copy(ot_sb, acc)
                if j == 0:
                    deferred0.append((h, ot_sb))
                elif last:
                    for fn in pending:
                        fn()
                    pending = []
                    for fn in phase2_units(j, h, ot_sb, last=True):
                        fn()
                else:
                    pending.extend(phase2_units(j, h, ot_sb))

    nc.compile()
    return nc


@functools.lru_cache(maxsize=2)
def _graph(use_sigmoid: bool):
    return _build(use_sigmoid)


def _shard(a: np.ndarray, i: int) -> np.ndarray:
    b, hg = divmod(i, 2)
    return np.ascontiguousarray(a[b, :, hg * CW : (hg + 1) * CW], dtype=np.float32)


def run(inputs, trace: bool = False):
    use_sigmoid = bool(np.asarray(inputs["use_sigmoid"]).item())
    nc = _graph(use_sigmoid)
    in_maps = []
    for i in range(8):
        m = {
            "q": _shard(np.asarray(inputs["query"]), i),
            "k": _shard(np.asarray(inputs["key"]), i),
            "v": _shard(np.asarray(inputs["value"]), i),
        }
        if use_sigmoid:
            m["q2"] = _shard(np.asarray(inputs["query2"]), i)
            m["k2"] = _shard(np.asarray(inputs["key2"]), i)
        in_maps.append(m)
    res = bass_utils.run_bass_kernel_spmd(
        nc, in_maps, core_ids=list(range(8)), trace=trace
    )
    out = np.empty((B, NT, C), dtype=np.float32)
    for i in range(8):
        b, hg = divmod(i, 2)
        out[b, :, hg * CW : (hg + 1) * CW] = res.results[i]["out"]
    return out, res


def kernel(**inputs) -> np.ndarray:
    out, _ = run(inputs)
    return out


if __name__ == "__main__":
    rng = np.random.default_rng(0)
    fake = {
        "query": rng.standard_normal((B, NT, C), dtype=np.float32),
        "key": rng.standard_normal((B, NT, C), dtype=np.float32),
        "value": rng.standard_normal((B, NT, C), dtype=np.float32),
        "query2": rng.standard_normal((B, NT, C), dtype=np.float32),
        "key2": rng.standard_normal((B, NT, C), dtype=np.float32),
        "use_sigmoid": 1,
    }
    out = kernel(**fake)
    print("ran ok", out.shape, out.dtype)

